# revision 1
# baseline (speedup 1.0000x reference)
"""Causal self-attention Bass/Tile kernel for Trainium2, data-parallel over batch.

Problem (hardcoded): x [8, 1024, 1024] f32, w_qkv [1024, 3072], w_proj [1024, 1024],
16 heads, dk=64, causal mask (masked_fill -10000), softmax, y = attn @ w_proj.

Sharding: batch 8 -> one batch element per NeuronCore (8 cores), weights
replicated, no collectives. QKV/proj matmuls in float32r (~tf32 inputs, fp32
accumulate; measured same PE rate as bf16 at N=512); attention operands
(K^T, Q^T, E=exp(S), V) in bf16 with fp32 accumulate - bf16 avoids the
fp32r 4x penalty on narrow (<256 col) causal-diagonal chunks.

Per-core dataflow (x is fed pre-transposed as xT [c, t]):
  A. xT and w_v stream in per-128-block as balanced pairs across both HWDGE
     queues so the V = x @ w_v chains start as blocks land. V stored bf16 at stride 96 per
     head: [V_h(64) | ones | ones-pad(31)] ([128, 8, 1536]): the M=128 PV
     stationary window then yields the softmax denominator as an extra
     output row of the same matmul - no separate denominator chains.
  B. Software-pipelined pack loop (2 heads per pack): S(pk) -> K/Q-gen(pk+1)
     -> PV(pk), so the PE fills the exp latency of pack pk with dense K/Q
     chains prefetched one pack ahead. S^T[k, q] matmuls (K=64, causal
     blocks only) of the head pair are emitted back-to-back so the two
     64-row strips execute concurrently on the PE sub-arrays. The
     diag-block causal mask is accumulated on the PE (psum += I^T @ mask)
     instead of a DVE add on PSUM. exp on ScalarE -> E tiles (bf16).
     PV stationary windows: even head starts at 96h -> y rows 0:64, dn row
     64; odd head starts at 96h-64 -> y rows 64:128, dn row 32 (the
     previous head's ones column; PSUM partition offsets must be
     32-aligned) - both heads' normalized outputs are written to their
     YT_all partitions directly by the DVE (no SBUF->SBUF DMA shift).
     1/dn: DVE reciprocal of the dn row into a partition-0 tile, broadcast
     to 128 rows on the idle GpSimd engine (its ucode reads physical
     partition 0), multiply on DVE.
  C. proj: y = YT-stationary @ w_proj 512-chunks -> natural [t, j] output;
     w_proj preloaded during early phase B; 6-deep staging ring and output
     DMAs alternating the two HWDGE queues keep the kernel tail short.
"""

import numpy as np

T = 1024
C = 1024
H = 16
DK = 64
NCORES = 8
P = 128
NEG_MASK = -1.0e9  # added pre-scale; exp(0.125 * (s + NEG_MASK)) == 0.0
VS = 96  # per-head V stride: 64 channels + ones col (64) + pad (65:96)
VCOLS = H * VS  # 1536

_CACHE = {}

# E-tile column offsets: block j occupies [EOFF[j], EOFF[j] + 1024 - 128*j)
EOFF = [1024 * j - 64 * j * (j - 1) for j in range(9)]  # EOFF[8] == 4608


def _build_program(loop_n=None, phases="ABC"):
    import concourse.tile as tile
    from concourse import bacc, mybir
    from contextlib import ExitStack

    F32R = mybir.dt.float32r
    F32 = mybir.dt.float32
    BF16 = mybir.dt.bfloat16

    nc = bacc.Bacc("TRN2", target_bir_lowering=False, debug=False, num_devices=NCORES)
    xt_d = nc.dram_tensor("xT", [C, T], F32R, kind="ExternalInput").ap()
    wqkv_d = nc.dram_tensor("w_qkv", [C, 3 * C], F32R, kind="ExternalInput").ap()
    wproj_d = nc.dram_tensor("w_proj", [C, C], F32R, kind="ExternalInput").ap()
    mask_d = nc.dram_tensor("mask", [P, P], BF16, kind="ExternalInput").ap()
    idb_d = nc.dram_tensor("idb", [P, P], BF16, kind="ExternalInput").ap()
    onesb_d = nc.dram_tensor("onesb", [P, 512], BF16, kind="ExternalInput").ap()
    y_d = nc.dram_tensor("y", [T, C], F32, kind="ExternalOutput").ap()

    NT = T // P  # 8 t/k blocks
    NCO = C // P  # 8 contraction blocks

    with tile.TileContext(nc) as tc:
        with ExitStack() as _st:
            if loop_n is not None:
                _st.enter_context(tc.For_i(0, loop_n, 1))
            _run_body(nc, tc, mybir, locals(), phases)
    nc.compile()
    return nc


def _run_body(nc, tc, mybir, env, phases="ABC"):
    import concourse.tile as tile

    F32R = mybir.dt.float32r
    F32 = mybir.dt.float32
    BF16 = mybir.dt.bfloat16
    EXP = mybir.ActivationFunctionType.Exp
    xt_d, wqkv_d, wproj_d = env["xt_d"], env["wqkv_d"], env["wproj_d"]
    mask_d, onesb_d, y_d = env["mask_d"], env["onesb_d"], env["y_d"]
    idb_d = env["idb_d"]
    NT, NCO = env["NT"], env["NCO"]
    if True:
        with (
            tc.tile_pool(name="consts", bufs=1) as consts,
            tc.tile_pool(name="persist", bufs=1) as persist,
            tc.tile_pool(name="psum", bufs=4, space="PSUM") as psum,
            tc.tile_pool(name="ytdn", bufs=4, space="PSUM") as ytdn,
            tc.tile_pool(name="xpool", bufs=1) as xpool,
        ):
            mask = consts.tile([P, P], BF16, tag="mask", name="mask")
            idb = consts.tile([P, P], BF16, tag="idb", name="idb")
            onesb = consts.tile([P, 512], BF16, tag="onesb", name="onesb")
            # proj weight chunks (DMA'd during early phase B)
            wp0 = consts.tile([P, NCO, 512], F32R, tag="wp0", name="wp0")
            wp1 = consts.tile([P, NCO, 512], F32R, tag="wp1", name="wp1")

            V_all = persist.tile([P, NT, VCOLS], BF16, tag="V", name="V_all")
            YT_all = persist.tile([P, NCO, T], F32R, tag="YT", name="YT_all")
            xT = xpool.tile([P, NCO, T], F32R, tag="xT", name="xT_all")

            # ---- Phase A: xT load; V projection ----
            with tc.tile_pool(name="wvp", bufs=2) as wvp:
                wv0 = wvp.tile([P, NCO, 512], F32R, tag="wv", name="wv0")
                wv0r = wqkv_d[:, 2 * C : 2 * C + 512].rearrange(
                    "(co p) j -> p co j", p=P
                )
                # balanced pair feed: even-co xT + odd-co wv on SP, odd-co
                # xT + even-co wv on Activation, so (xT co, wv co) pairs land
                # in co order at the two queues' combined bandwidth
                for co in range(NCO):
                    xe = nc.sync if co % 2 == 0 else nc.scalar
                    we = nc.scalar if co % 2 == 0 else nc.sync
                    xe.dma_start(xT[:, co, :], xt_d[co * P : (co + 1) * P, :])
                    we.dma_start(wv0[:, co, :], wv0r[:, co, :])
                # consts trail the startup-critical loads (first use ~60us)
                nc.sync.dma_start(mask[:], mask_d)
                nc.sync.dma_start(idb[:], idb_d)
                nc.sync.dma_start(onesb[:], onesb_d)
                for j in range(NT):
                    # ones + pad columns (local cols 64:96 of each head block)
                    nc.vector.tensor_copy(
                        V_all[:, j, :].rearrange("p (h c) -> p h c", c=VS)[
                            :, 0:H, DK:VS
                        ],
                        onesb[:].rearrange("p (h c) -> p h c", c=VS - DK),
                    )
                for jc in range(2):  # 512-wide w_v column chunks
                    if jc == 0:
                        wv = wv0
                    else:
                        wv = wvp.tile([P, NCO, 512], F32R, tag="wv", name="wv")
                        wvr = wqkv_d[
                            :, 2 * C + jc * 512 : 2 * C + (jc + 1) * 512
                        ].rearrange("(co p) j -> p co j", p=P)
                        for co in range(NCO):
                            eng = nc.scalar if co % 2 == 0 else nc.sync
                            eng.dma_start(wv[:, co, :], wvr[:, co, :])
                    for tb in range(NT):
                        ps = psum.tile([P, 512], F32, tag="mm", name="psv")
                        for co in range(NCO):
                            nc.tensor.matmul(
                                ps[:],
                                xT[:, co, tb * P : (tb + 1) * P],
                                wv[:, co, :],
                                start=(co == 0),
                                stop=(co == NCO - 1),
                            )
                        # scatter 8 heads of 64 channels into stride-96 slots
                        nc.vector.tensor_copy(
                            V_all[:, tb, :].rearrange("p (h c) -> p h c", c=VS)[
                                :, jc * 8 : (jc + 1) * 8, 0:DK
                            ],
                            ps[:].rearrange("p (h c) -> p h c", c=DK),
                        )

            # ---- Phase B: software-pipelined pack loop ----
            NPK = H // 2 if "B" in phases else 0
            with tc.tile_pool(name="attp", bufs=2) as attp, \
                 tc.tile_pool(name="ktqt", bufs=3) as ktqt, \
                 tc.tile_pool(name="epool", bufs=4) as epool:

                def emit_kq(pk):
                    packs = {}
                    for key, base in (("kt", C), ("qt", 0)):
                        wq = attp.tile([P, NCO, P], F32R, tag="wqk", name="wqk")
                        wqr = wqkv_d[
                            :, base + P * pk : base + P * (pk + 1)
                        ].rearrange("(co p) j -> p co j", p=P)
                        nc.sync.dma_start(wq[:], wqr[:, :, :])
                        dst = ktqt.tile([P, T], BF16, tag=key, name=key)
                        for th in range(2):
                            ps = psum.tile([P, 512], F32, tag="mm", name="psq")
                            for co in range(NCO):
                                nc.tensor.matmul(
                                    ps[:],
                                    wq[:, co, :],
                                    xT[:, co, th * 512 : (th + 1) * 512],
                                    start=(co == 0),
                                    stop=(co == NCO - 1),
                                )
                            nc.vector.tensor_copy(
                                dst[:, th * 512 : (th + 1) * 512], ps[:]
                            )
                        packs[key] = dst
                    return packs["kt"], packs["qt"]

                kqs = {}
                if NPK:
                    kqs[0] = emit_kq(0)
                for pk in range(NPK):
                    kt, qt = kqs.pop(pk)
                    # S^T + exp for both heads, S matmuls of the pair emitted
                    # back-to-back: the two K=64 row strips (partitions 0:64
                    # and 64:128) execute concurrently on the PE sub-arrays.
                    ets = [
                        epool.tile([P, EOFF[NT]], BF16, tag="e", name="et0"),
                        epool.tile([P, EOFF[NT]], BF16, tag="e", name="et1"),
                    ]
                    for j in range(NT):
                        q0 = j * P
                        for cs in range(q0, T, 512):
                            cw = min(512, T - cs)
                            diag = cs == q0
                            sps = []
                            for r in range(2):
                                R = slice(DK * r, DK * (r + 1))
                                sp = psum.tile([P, 512], F32, tag="mm", name="sp")
                                nc.tensor.matmul(
                                    sp[:, :cw],
                                    kt[R, q0 : q0 + P],
                                    qt[R, cs : cs + cw],
                                    start=True,
                                    stop=not diag,
                                )
                                if diag:
                                    # causal mask of the diagonal block added
                                    # on the PE: psum += I^T @ mask
                                    nc.tensor.matmul(
                                        sp[:, :P],
                                        idb[:],
                                        mask[:],
                                        start=False,
                                        stop=True,
                                    )
                                sps.append(sp)
                            for r in range(2):
                                sp = sps[r]
                                o = EOFF[j] + cs - q0
                                nc.scalar.activation(
                                    ets[r][:, o : o + cw], sp[:, :cw], EXP,
                                    bias=0.0, scale=0.125,
                                )
                    if pk == 0:
                        # proj weights: queued behind pack-1 K/Q weights,
                        # long before phase C needs them
                        nc.scalar.dma_start(
                            wp0[:],
                            wproj_d[:, 0:512].rearrange("(co p) j -> p co j", p=P),
                        )
                        nc.scalar.dma_start(
                            wp1[:],
                            wproj_d[:, 512:1024].rearrange("(co p) j -> p co j", p=P),
                        )
                    if pk + 1 < NPK:
                        kqs[pk + 1] = emit_kq(pk + 1)
                    for r in range(2):
                        h = 2 * pk + r
                        et = ets[r]
                        # PV with M=128 stationary window; even head
                        # [V_h | 1 | pad..] -> y rows 0:64, dn row 64;
                        # odd head shifted -64 [.. | 1(h-1) | pad | V_h] ->
                        # y rows 64:128, dn row 32 (ones col of head h-1,
                        # 32-aligned as PSUM partition offsets require)
                        wst = VS * h if r == 0 else VS * h - DK
                        dnr = DK if r == 0 else DK - 32
                        ra, rb = (0, DK) if r == 0 else (DK, P)
                        for ck in range(2):
                            a0 = ck * 512
                            b = (ck + 1) * 512
                            ytc = ytdn.tile([P, 512], F32, tag="yt", name="ytc")
                            jmax = min(NT - 1, (b - 1) // P)
                            for j in range(jmax + 1):
                                q0 = j * P
                                a = max(q0, a0)
                                esl = et[:, EOFF[j] + a - q0 : EOFF[j] + b - q0]
                                nc.tensor.matmul(
                                    ytc[:, a - a0 : b - a0],
                                    V_all[:, j, wst : wst + P],
                                    esl,
                                    start=(j == 0),
                                    stop=(j == jmax),
                                    skip_group_check=True,
                                )
                            # 1/dn from PSUM row dnr into a partition-0 tile,
                            # broadcast to all 128 rows on GpSimd (the ucode
                            # broadcasts physical partition 0), normalize ->
                            # YT pack rows
                            rec1 = attp.tile([1, 512], F32, tag="rec1", name="rec1")
                            nc.vector.reciprocal(rec1[:], ytc[dnr : dnr + 1, :])
                            rec = attp.tile([P, 512], F32, tag="rec", name="rec")
                            nc.gpsimd.partition_broadcast(rec[:], rec1[:])
                            nc.vector.tensor_mul(
                                YT_all[ra:rb, pk, a0:b],
                                ytc[ra:rb, :],
                                rec[ra:rb, :],
                            )

            # ---- Phase C: output projection ----
            with tc.tile_pool(name="projp", bufs=6) as projp:
                for jc in range(2 if "C" in phases else 0):
                    wp = wp0 if jc == 0 else wp1
                    for tb in range(NT):
                        ps = psum.tile([P, 512], F32, tag="mm", name="pso")
                        for co in range(NCO):
                            nc.tensor.matmul(
                                ps[:],
                                YT_all[:, co, tb * P : (tb + 1) * P],
                                wp[:, co, :],
                                start=(co == 0),
                                stop=(co == NCO - 1),
                            )
                        # deep staging ring so the copy->DMA tail of the last
                        # chunks pipelines instead of serializing the drain
                        ot = projp.tile([P, 512], F32, tag="ot", name="ot")
                        nc.vector.tensor_copy(ot[:], ps[:])
                        eng = nc.sync if tb % 2 == 0 else nc.scalar
                        eng.dma_start(
                            y_d[tb * P : (tb + 1) * P, jc * 512 : (jc + 1) * 512],
                            ot[:],
                        )


def _get_program():
    if "nc" not in _CACHE:
        _CACHE["nc"] = _build_program()
    return _CACHE["nc"]


def make_in_maps(x, w_qkv, w_proj):
    import ml_dtypes

    x = np.asarray(x, dtype=np.float32)
    xT = np.ascontiguousarray(x.transpose(0, 2, 1))
    w_qkv = np.ascontiguousarray(np.asarray(w_qkv), dtype=np.float32)
    w_proj = np.ascontiguousarray(np.asarray(w_proj), dtype=np.float32)
    # S^T[k, q] diag-block mask: keep q >= k (upper triangle incl. diagonal)
    mask = np.where(
        np.arange(P)[None, :] >= np.arange(P)[:, None], 0.0, NEG_MASK
    ).astype(ml_dtypes.bfloat16)
    idb = np.eye(P, dtype=ml_dtypes.bfloat16)
    onesb = np.ones((P, 512), dtype=ml_dtypes.bfloat16)
    return [
        {
            "xT": xT[i],
            "w_qkv": w_qkv,
            "w_proj": w_proj,
            "mask": mask,
            "idb": idb,
            "onesb": onesb,
        }
        for i in range(NCORES)
    ]


def kernel(x, w_qkv, w_proj, _trace=False):
    from concourse.bass_utils import run_bass_kernel_spmd

    nc = _get_program()
    in_maps = make_in_maps(x, w_qkv, w_proj)
    res = run_bass_kernel_spmd(nc, in_maps, list(range(NCORES)), trace=_trace)
    out = np.stack([res.results[i]["y"] for i in range(NCORES)], axis=0)
    if _trace:
        return out, res
    return out



# revision 3
# speedup vs baseline: 3.9797x; 3.9797x over previous
"""Causal self-attention Bass/Tile kernel for Trainium2, data-parallel over batch.

Problem (hardcoded): x [8, 1024, 1024] f32, w_qkv [1024, 3072], w_proj [1024, 1024],
16 heads, dk=64, causal mask (masked_fill -10000), softmax, y = attn @ w_proj.

Sharding: batch 8 -> one batch element per NeuronCore (8 cores), weights
replicated, no collectives. QKV/proj matmuls in float32r (~tf32 inputs, fp32
accumulate; measured same PE rate as bf16 at N=512); attention operands
(K^T, Q^T, E=exp(S), V) in bf16 with fp32 accumulate - bf16 avoids the
fp32r 4x penalty on narrow (<256 col) causal-diagonal chunks.

Per-core dataflow (x is fed pre-transposed as xT [c, t]):
  A. xT and w_v stream in per-128-block as balanced pairs across both HWDGE
     queues so the V = x @ w_v chains start as blocks land. V stored bf16 at stride 96 per
     head: [V_h(64) | ones | ones-pad(31)] ([128, 8, 1536]): the M=128 PV
     stationary window then yields the softmax denominator as an extra
     output row of the same matmul - no separate denominator chains.
  B. Software-pipelined pack loop (2 heads per pack): S(pk) -> K/Q-gen(pk+1)
     -> PV(pk), so the PE fills the exp latency of pack pk with dense K/Q
     chains prefetched one pack ahead. S^T[k, q] matmuls (K=64, causal
     blocks only) of the head pair are emitted back-to-back so the two
     64-row strips execute concurrently on the PE sub-arrays. The
     diag-block causal mask is accumulated on the PE (psum += I^T @ mask)
     instead of a DVE add on PSUM. exp on ScalarE -> E tiles (bf16).
     PV stationary windows: even head starts at 96h -> y rows 0:64, dn row
     64; odd head starts at 96h-64 -> y rows 64:128, dn row 32 (the
     previous head's ones column; PSUM partition offsets must be
     32-aligned) - both heads' normalized outputs are written to their
     YT_all partitions directly by the DVE (no SBUF->SBUF DMA shift).
     1/dn: DVE reciprocal of the dn row into a partition-0 tile, broadcast
     to 128 rows on the idle GpSimd engine (its ucode reads physical
     partition 0), multiply on DVE.
  C. proj: y = YT-stationary @ w_proj 512-chunks -> natural [t, j] output;
     w_proj preloaded during early phase B; 6-deep staging ring and output
     DMAs alternating the two HWDGE queues keep the kernel tail short.
"""

import numpy as np

T = 1024
C = 1024
H = 16
DK = 64
NCORES = 8
P = 128
NEG_MASK = -1.0e9  # added pre-scale; exp(0.125 * (s + NEG_MASK)) == 0.0
VS = 96  # per-head V stride: 64 channels + ones col (64) + pad (65:96)
VCOLS = H * VS  # 1536

_CACHE = {}

# E-tile column offsets: block j occupies [EOFF[j], EOFF[j] + 1024 - 128*j)
EOFF = [1024 * j - 64 * j * (j - 1) for j in range(9)]  # EOFF[8] == 4608


def _build_program(loop_n=None, phases="ABC"):
    import concourse.tile as tile
    from concourse import bacc, mybir
    from contextlib import ExitStack

    F32R = mybir.dt.float32r
    F32 = mybir.dt.float32
    BF16 = mybir.dt.bfloat16

    nc = bacc.Bacc("TRN2", target_bir_lowering=False, debug=False, num_devices=NCORES)
    xt_d = nc.dram_tensor("xT", [C, T], F32R, kind="ExternalInput").ap()
    wqkv_d = nc.dram_tensor("w_qkv", [C, 3 * C], F32R, kind="ExternalInput").ap()
    wproj_d = nc.dram_tensor("w_proj", [C, C], F32R, kind="ExternalInput").ap()
    mask_d = nc.dram_tensor("mask", [P, P], BF16, kind="ExternalInput").ap()
    idb_d = nc.dram_tensor("idb", [P, P], BF16, kind="ExternalInput").ap()
    onesb_d = nc.dram_tensor("onesb", [P, 512], BF16, kind="ExternalInput").ap()
    y_d = nc.dram_tensor("y", [T, C], F32, kind="ExternalOutput").ap()

    NT = T // P  # 8 t/k blocks
    NCO = C // P  # 8 contraction blocks

    with tile.TileContext(nc) as tc:
        with ExitStack() as _st:
            if loop_n is not None:
                _st.enter_context(tc.For_i(0, loop_n, 1))
            _run_body(nc, tc, mybir, locals(), phases)
    nc.compile()
    return nc


def _run_body(nc, tc, mybir, env, phases="ABC"):
    import concourse.tile as tile

    F32R = mybir.dt.float32r
    F32 = mybir.dt.float32
    BF16 = mybir.dt.bfloat16
    EXP = mybir.ActivationFunctionType.Exp
    xt_d, wqkv_d, wproj_d = env["xt_d"], env["wqkv_d"], env["wproj_d"]
    mask_d, onesb_d, y_d = env["mask_d"], env["onesb_d"], env["y_d"]
    idb_d = env["idb_d"]
    NT, NCO = env["NT"], env["NCO"]
    if True:
        with (
            tc.tile_pool(name="consts", bufs=1) as consts,
            tc.tile_pool(name="persist", bufs=1) as persist,
            tc.tile_pool(name="psum", bufs=4, space="PSUM") as psum,
            tc.tile_pool(name="ytdn", bufs=4, space="PSUM") as ytdn,
            tc.tile_pool(name="xpool", bufs=1) as xpool,
        ):
            mask = consts.tile([P, P], BF16, tag="mask", name="mask")
            idb = consts.tile([P, P], BF16, tag="idb", name="idb")
            onesb = consts.tile([P, 512], BF16, tag="onesb", name="onesb")
            # proj weight chunks (DMA'd during early phase B)
            wp0 = consts.tile([P, NCO, 512], F32R, tag="wp0", name="wp0")
            wp1 = consts.tile([P, NCO, 512], F32R, tag="wp1", name="wp1")

            V_all = persist.tile([P, NT, VCOLS], BF16, tag="V", name="V_all")
            YT_all = persist.tile([P, NCO, T], F32R, tag="YT", name="YT_all")
            xT = xpool.tile([P, NCO, T], F32R, tag="xT", name="xT_all")

            # ---- Phase A: xT load; V projection ----
            with tc.tile_pool(name="wvp", bufs=2) as wvp:
                wv0 = wvp.tile([P, NCO, 512], F32R, tag="wv", name="wv0")
                wv0r = wqkv_d[:, 2 * C : 2 * C + 512].rearrange(
                    "(co p) j -> p co j", p=P
                )
                # balanced pair feed: even-co xT + odd-co wv on SP, odd-co
                # xT + even-co wv on Activation, so (xT co, wv co) pairs land
                # in co order at the two queues' combined bandwidth
                for co in range(NCO):
                    xe = nc.sync if co % 2 == 0 else nc.scalar
                    we = nc.scalar if co % 2 == 0 else nc.sync
                    xe.dma_start(xT[:, co, :], xt_d[co * P : (co + 1) * P, :])
                    we.dma_start(wv0[:, co, :], wv0r[:, co, :])
                # consts trail the startup-critical loads (first use ~60us)
                nc.sync.dma_start(mask[:], mask_d)
                nc.sync.dma_start(idb[:], idb_d)
                nc.sync.dma_start(onesb[:], onesb_d)
                for j in range(NT):
                    # ones + pad columns (local cols 64:96 of each head block)
                    nc.vector.tensor_copy(
                        V_all[:, j, :].rearrange("p (h c) -> p h c", c=VS)[
                            :, 0:H, DK:VS
                        ],
                        onesb[:].rearrange("p (h c) -> p h c", c=VS - DK),
                    )
                for jc in range(2):  # 512-wide w_v column chunks
                    if jc == 0:
                        wv = wv0
                    else:
                        wv = wvp.tile([P, NCO, 512], F32R, tag="wv", name="wv")
                        wvr = wqkv_d[
                            :, 2 * C + jc * 512 : 2 * C + (jc + 1) * 512
                        ].rearrange("(co p) j -> p co j", p=P)
                        for co in range(NCO):
                            eng = nc.scalar if co % 2 == 0 else nc.sync
                            eng.dma_start(wv[:, co, :], wvr[:, co, :])
                    for tb in range(NT):
                        ps = psum.tile([P, 512], F32, tag="mm", name="psv")
                        for co in range(NCO):
                            nc.tensor.matmul(
                                ps[:],
                                xT[:, co, tb * P : (tb + 1) * P],
                                wv[:, co, :],
                                start=(co == 0),
                                stop=(co == NCO - 1),
                            )
                        # scatter 8 heads of 64 channels into stride-96 slots
                        nc.vector.tensor_copy(
                            V_all[:, tb, :].rearrange("p (h c) -> p h c", c=VS)[
                                :, jc * 8 : (jc + 1) * 8, 0:DK
                            ],
                            ps[:].rearrange("p (h c) -> p h c", c=DK),
                        )

            # ---- Phase B: software-pipelined pack loop ----
            NPK = H // 2 if "B" in phases else 0
            with tc.tile_pool(name="attp", bufs=2) as attp, \
                 tc.tile_pool(name="ktqt", bufs=3) as ktqt, \
                 tc.tile_pool(name="epool", bufs=4) as epool:

                def emit_kq(pk):
                    packs = {}
                    for key, base in (("kt", C), ("qt", 0)):
                        wq = attp.tile([P, NCO, P], F32R, tag="wqk", name="wqk")
                        wqr = wqkv_d[
                            :, base + P * pk : base + P * (pk + 1)
                        ].rearrange("(co p) j -> p co j", p=P)
                        nc.sync.dma_start(wq[:], wqr[:, :, :])
                        dst = ktqt.tile([P, T], BF16, tag=key, name=key)
                        for th in range(2):
                            ps = psum.tile([P, 512], F32, tag="mm", name="psq")
                            for co in range(NCO):
                                nc.tensor.matmul(
                                    ps[:],
                                    wq[:, co, :],
                                    xT[:, co, th * 512 : (th + 1) * 512],
                                    start=(co == 0),
                                    stop=(co == NCO - 1),
                                )
                            nc.vector.tensor_copy(
                                dst[:, th * 512 : (th + 1) * 512], ps[:]
                            )
                        packs[key] = dst
                    return packs["kt"], packs["qt"]

                kqs = {}
                if NPK:
                    kqs[0] = emit_kq(0)
                for pk in range(NPK):
                    kt, qt = kqs.pop(pk)
                    # S^T + exp for both heads, S matmuls of the pair emitted
                    # back-to-back: the two K=64 row strips (partitions 0:64
                    # and 64:128) execute concurrently on the PE sub-arrays.
                    ets = [
                        epool.tile([P, EOFF[NT]], BF16, tag="e", name="et0"),
                        epool.tile([P, EOFF[NT]], BF16, tag="e", name="et1"),
                    ]
                    for j in range(NT):
                        q0 = j * P
                        for cs in range(q0, T, 512):
                            cw = min(512, T - cs)
                            diag = cs == q0
                            sps = []
                            for r in range(2):
                                R = slice(DK * r, DK * (r + 1))
                                sp = psum.tile([P, 512], F32, tag="mm", name="sp")
                                nc.tensor.matmul(
                                    sp[:, :cw],
                                    kt[R, q0 : q0 + P],
                                    qt[R, cs : cs + cw],
                                    start=True,
                                    stop=not diag,
                                )
                                if diag:
                                    # causal mask of the diagonal block added
                                    # on the PE: psum += I^T @ mask
                                    nc.tensor.matmul(
                                        sp[:, :P],
                                        idb[:],
                                        mask[:],
                                        start=False,
                                        stop=True,
                                    )
                                sps.append(sp)
                            for r in range(2):
                                sp = sps[r]
                                o = EOFF[j] + cs - q0
                                nc.scalar.activation(
                                    ets[r][:, o : o + cw], sp[:, :cw], EXP,
                                    bias=0.0, scale=0.125,
                                )
                    if pk == 0:
                        # proj weights: queued behind pack-1 K/Q weights,
                        # long before phase C needs them
                        nc.scalar.dma_start(
                            wp0[:],
                            wproj_d[:, 0:512].rearrange("(co p) j -> p co j", p=P),
                        )
                        nc.scalar.dma_start(
                            wp1[:],
                            wproj_d[:, 512:1024].rearrange("(co p) j -> p co j", p=P),
                        )
                    if pk + 1 < NPK:
                        kqs[pk + 1] = emit_kq(pk + 1)
                    for r in range(2):
                        h = 2 * pk + r
                        et = ets[r]
                        # PV with M=128 stationary window; even head
                        # [V_h | 1 | pad..] -> y rows 0:64, dn row 64;
                        # odd head shifted -64 [.. | 1(h-1) | pad | V_h] ->
                        # y rows 64:128, dn row 32 (ones col of head h-1,
                        # 32-aligned as PSUM partition offsets require)
                        wst = VS * h if r == 0 else VS * h - DK
                        dnr = DK if r == 0 else DK - 32
                        ra, rb = (0, DK) if r == 0 else (DK, P)
                        for ck in range(2):
                            a0 = ck * 512
                            b = (ck + 1) * 512
                            ytc = ytdn.tile([P, 512], F32, tag="yt", name="ytc")
                            jmax = min(NT - 1, (b - 1) // P)
                            for j in range(jmax + 1):
                                q0 = j * P
                                a = max(q0, a0)
                                esl = et[:, EOFF[j] + a - q0 : EOFF[j] + b - q0]
                                nc.tensor.matmul(
                                    ytc[:, a - a0 : b - a0],
                                    V_all[:, j, wst : wst + P],
                                    esl,
                                    start=(j == 0),
                                    stop=(j == jmax),
                                    skip_group_check=True,
                                )
                            # 1/dn from PSUM row dnr into a partition-0 tile,
                            # broadcast to all 128 rows on GpSimd (the ucode
                            # broadcasts physical partition 0), normalize ->
                            # YT pack rows
                            dnsb = attp.tile([1, 512], F32, tag="dnsb", name="dnsb")
                            nc.vector.tensor_copy(dnsb[:], ytc[dnr : dnr + 1, :])
                            rec1 = attp.tile([1, 512], F32, tag="rec1", name="rec1")
                            nc.vector.reciprocal_approx_fast(rec1[:], dnsb[:])
                            rec = attp.tile([P, 512], F32, tag="rec", name="rec")
                            nc.gpsimd.partition_broadcast(rec[:], rec1[:])
                            nc.vector.tensor_mul(
                                YT_all[ra:rb, pk, a0:b],
                                ytc[ra:rb, :],
                                rec[ra:rb, :],
                            )

            # ---- Phase C: output projection ----
            with tc.tile_pool(name="projp", bufs=6) as projp:
                for jc in range(2 if "C" in phases else 0):
                    wp = wp0 if jc == 0 else wp1
                    for tb in range(NT):
                        ps = psum.tile([P, 512], F32, tag="mm", name="pso")
                        for co in range(NCO):
                            nc.tensor.matmul(
                                ps[:],
                                YT_all[:, co, tb * P : (tb + 1) * P],
                                wp[:, co, :],
                                start=(co == 0),
                                stop=(co == NCO - 1),
                            )
                        # deep staging ring so the copy->DMA tail of the last
                        # chunks pipelines instead of serializing the drain
                        ot = projp.tile([P, 512], F32, tag="ot", name="ot")
                        nc.vector.tensor_copy(ot[:], ps[:])
                        eng = nc.sync if tb % 2 == 0 else nc.scalar
                        eng.dma_start(
                            y_d[tb * P : (tb + 1) * P, jc * 512 : (jc + 1) * 512],
                            ot[:],
                        )


def _get_program():
    if "nc" not in _CACHE:
        _CACHE["nc"] = _build_program()
    return _CACHE["nc"]


def make_in_maps(x, w_qkv, w_proj):
    import ml_dtypes

    x = np.asarray(x, dtype=np.float32)
    xT = np.ascontiguousarray(x.transpose(0, 2, 1))
    w_qkv = np.ascontiguousarray(np.asarray(w_qkv), dtype=np.float32)
    w_proj = np.ascontiguousarray(np.asarray(w_proj), dtype=np.float32)
    # S^T[k, q] diag-block mask: keep q >= k (upper triangle incl. diagonal)
    mask = np.where(
        np.arange(P)[None, :] >= np.arange(P)[:, None], 0.0, NEG_MASK
    ).astype(ml_dtypes.bfloat16)
    idb = np.eye(P, dtype=ml_dtypes.bfloat16)
    onesb = np.ones((P, 512), dtype=ml_dtypes.bfloat16)
    return [
        {
            "xT": xT[i],
            "w_qkv": w_qkv,
            "w_proj": w_proj,
            "mask": mask,
            "idb": idb,
            "onesb": onesb,
        }
        for i in range(NCORES)
    ]


def kernel(x, w_qkv, w_proj, _trace=False):
    from concourse.bass_utils import run_bass_kernel_spmd

    nc = _get_program()
    in_maps = make_in_maps(x, w_qkv, w_proj)
    res = run_bass_kernel_spmd(nc, in_maps, list(range(NCORES)), trace=_trace)
    out = np.stack([res.results[i]["y"] for i in range(NCORES)], axis=0)
    if _trace:
        return out, res
    return out



# revision 5
# speedup vs baseline: 4.2501x; 1.0680x over previous
"""Causal self-attention Bass/Tile kernel for Trainium2, data-parallel over batch.

Problem (hardcoded): x [8, 1024, 1024] f32, w_qkv [1024, 3072], w_proj [1024, 1024],
16 heads, dk=64, causal mask (masked_fill -10000), softmax, y = attn @ w_proj.

Sharding: batch 8 -> one batch element per NeuronCore (8 cores), weights
replicated, no collectives. QKV/proj matmuls in float32r (~tf32 inputs, fp32
accumulate; measured same PE rate as bf16 at N=512); attention operands
(K^T, Q^T, E=exp(S), V) in bf16 with fp32 accumulate - bf16 avoids the
fp32r 4x penalty on narrow (<256 col) causal-diagonal chunks.

Per-core dataflow (x is fed pre-transposed as xT [c, t]):
  A. xT and w_v stream in per-128-block as balanced pairs across both HWDGE
     queues so the V = x @ w_v chains start as blocks land. V stored bf16 at stride 96 per
     head: [V_h(64) | ones | ones-pad(31)] ([128, 8, 1536]): the M=128 PV
     stationary window then yields the softmax denominator as an extra
     output row of the same matmul - no separate denominator chains.
  B. Software-pipelined pack loop (2 heads per pack): S(pk) -> K/Q-gen(pk+1)
     -> PV(pk), so the PE fills the exp latency of pack pk with dense K/Q
     chains prefetched one pack ahead. S^T[k, q] matmuls (K=64, causal
     blocks only) of the head pair are emitted back-to-back so the two
     64-row strips execute concurrently on the PE sub-arrays. The
     diag-block causal mask is accumulated on the PE (psum += I^T @ mask)
     instead of a DVE add on PSUM. exp on ScalarE -> E tiles (bf16).
     PV stationary windows: even head starts at 96h -> y rows 0:64, dn row
     64; odd head starts at 96h-64 -> y rows 64:128, dn row 32 (the
     previous head's ones column; PSUM partition offsets must be
     32-aligned) - both heads' normalized outputs are written to their
     YT_all partitions directly by the DVE (no SBUF->SBUF DMA shift).
     1/dn: DVE reciprocal of the dn row into a partition-0 tile, broadcast
     to 128 rows on the idle GpSimd engine (its ucode reads physical
     partition 0), multiply on DVE.
  C. proj: y = YT-stationary @ w_proj 512-chunks -> natural [t, j] output;
     w_proj preloaded during early phase B; 6-deep staging ring and output
     DMAs alternating the two HWDGE queues keep the kernel tail short.
"""

import numpy as np

T = 1024
C = 1024
H = 16
DK = 64
NCORES = 8
P = 128
NEG_MASK = -1.0e9  # added pre-scale; exp(0.125 * (s + NEG_MASK)) == 0.0
VS = 96  # per-head V stride: 64 channels + ones col (64) + pad (65:96)
VCOLS = H * VS  # 1536

_CACHE = {}

# E-tile column offsets: block j occupies [EOFF[j], EOFF[j] + 1024 - 128*j)
EOFF = [1024 * j - 64 * j * (j - 1) for j in range(9)]  # EOFF[8] == 4608


def _build_program(loop_n=None, phases="ABC"):
    import concourse.tile as tile
    from concourse import bacc, mybir
    from contextlib import ExitStack

    F32R = mybir.dt.float32r
    F32 = mybir.dt.float32
    BF16 = mybir.dt.bfloat16

    nc = bacc.Bacc("TRN2", target_bir_lowering=False, debug=False, num_devices=NCORES)
    xt_d = nc.dram_tensor("xT", [C, T], F32R, kind="ExternalInput").ap()
    wqkv_d = nc.dram_tensor("w_qkv", [C, 3 * C], F32R, kind="ExternalInput").ap()
    wproj_d = nc.dram_tensor("w_proj", [C, C], F32R, kind="ExternalInput").ap()
    mask_d = nc.dram_tensor("mask", [P, P], BF16, kind="ExternalInput").ap()
    idb_d = nc.dram_tensor("idb", [P, P], BF16, kind="ExternalInput").ap()
    onesb_d = nc.dram_tensor("onesb", [P, 512], BF16, kind="ExternalInput").ap()
    y_d = nc.dram_tensor("y", [T, C], F32, kind="ExternalOutput").ap()

    NT = T // P  # 8 t/k blocks
    NCO = C // P  # 8 contraction blocks

    with tile.TileContext(nc) as tc:
        with ExitStack() as _st:
            if loop_n is not None:
                _st.enter_context(tc.For_i(0, loop_n, 1))
            _run_body(nc, tc, mybir, locals(), phases)
    nc.compile()
    return nc


def _run_body(nc, tc, mybir, env, phases="ABC"):
    import concourse.tile as tile

    F32R = mybir.dt.float32r
    F32 = mybir.dt.float32
    BF16 = mybir.dt.bfloat16
    EXP = mybir.ActivationFunctionType.Exp
    xt_d, wqkv_d, wproj_d = env["xt_d"], env["wqkv_d"], env["wproj_d"]
    mask_d, onesb_d, y_d = env["mask_d"], env["onesb_d"], env["y_d"]
    idb_d = env["idb_d"]
    NT, NCO = env["NT"], env["NCO"]
    if True:
        with (
            tc.tile_pool(name="consts", bufs=1) as consts,
            tc.tile_pool(name="persist", bufs=1) as persist,
            tc.tile_pool(name="psum", bufs=4, space="PSUM") as psum,
            tc.tile_pool(name="ytdn", bufs=4, space="PSUM") as ytdn,
            tc.tile_pool(name="xpool", bufs=1) as xpool,
        ):
            mask = consts.tile([P, P], BF16, tag="mask", name="mask")
            idb = consts.tile([P, P], BF16, tag="idb", name="idb")
            onesb = consts.tile([P, 512], BF16, tag="onesb", name="onesb")
            # proj weight chunks (DMA'd during early phase B)
            wp0 = consts.tile([P, NCO, 512], F32R, tag="wp0", name="wp0")
            wp1 = consts.tile([P, NCO, 512], F32R, tag="wp1", name="wp1")

            V_all = persist.tile([P, NT, VCOLS], BF16, tag="V", name="V_all")
            YT_all = persist.tile([P, NCO, T], F32R, tag="YT", name="YT_all")
            xT = xpool.tile([P, NCO, T], F32R, tag="xT", name="xT_all")

            # ---- Phase A: xT load; V projection ----
            with tc.tile_pool(name="wvp", bufs=2) as wvp:
                wv0 = wvp.tile([P, NCO, 512], F32R, tag="wv", name="wv0")
                wv0r = wqkv_d[:, 2 * C : 2 * C + 512].rearrange(
                    "(co p) j -> p co j", p=P
                )
                # balanced pair feed: even-co xT + odd-co wv on SP, odd-co
                # xT + even-co wv on Activation, so (xT co, wv co) pairs land
                # in co order at the two queues' combined bandwidth
                for co in range(NCO):
                    xe = nc.sync if co % 2 == 0 else nc.scalar
                    we = nc.scalar if co % 2 == 0 else nc.sync
                    xe.dma_start(xT[:, co, :], xt_d[co * P : (co + 1) * P, :])
                    we.dma_start(wv0[:, co, :], wv0r[:, co, :])
                # consts trail the startup-critical loads (first use ~60us)
                nc.sync.dma_start(mask[:], mask_d)
                nc.sync.dma_start(idb[:], idb_d)
                nc.sync.dma_start(onesb[:], onesb_d)
                for j in range(NT):
                    # ones + pad columns (local cols 64:96 of each head block)
                    nc.vector.tensor_copy(
                        V_all[:, j, :].rearrange("p (h c) -> p h c", c=VS)[
                            :, 0:H, DK:VS
                        ],
                        onesb[:].rearrange("p (h c) -> p h c", c=VS - DK),
                    )
                for jc in range(2):  # 512-wide w_v column chunks
                    if jc == 0:
                        wv = wv0
                    else:
                        wv = wvp.tile([P, NCO, 512], F32R, tag="wv", name="wv")
                        wvr = wqkv_d[
                            :, 2 * C + jc * 512 : 2 * C + (jc + 1) * 512
                        ].rearrange("(co p) j -> p co j", p=P)
                        for co in range(NCO):
                            eng = nc.scalar if co % 2 == 0 else nc.sync
                            eng.dma_start(wv[:, co, :], wvr[:, co, :])
                    for tb in range(NT):
                        ps = psum.tile([P, 512], F32, tag="mm", name="psv")
                        for co in range(NCO):
                            nc.tensor.matmul(
                                ps[:],
                                xT[:, co, tb * P : (tb + 1) * P],
                                wv[:, co, :],
                                start=(co == 0),
                                stop=(co == NCO - 1),
                            )
                        # scatter 8 heads of 64 channels into stride-96 slots
                        nc.vector.tensor_copy(
                            V_all[:, tb, :].rearrange("p (h c) -> p h c", c=VS)[
                                :, jc * 8 : (jc + 1) * 8, 0:DK
                            ],
                            ps[:].rearrange("p (h c) -> p h c", c=DK),
                        )

            # ---- Phase B: software-pipelined pack loop ----
            NPK = H // 2 if "B" in phases else 0
            with tc.tile_pool(name="attp", bufs=2) as attp, \
                 tc.tile_pool(name="ktqt", bufs=3) as ktqt, \
                 tc.tile_pool(name="epool", bufs=4) as epool:

                def emit_kq(pk):
                    packs = {}
                    for key, base in (("kt", C), ("qt", 0)):
                        wq = attp.tile([P, NCO, P], F32R, tag="wqk", name="wqk")
                        wqr = wqkv_d[
                            :, base + P * pk : base + P * (pk + 1)
                        ].rearrange("(co p) j -> p co j", p=P)
                        nc.sync.dma_start(wq[:], wqr[:, :, :])
                        dst = ktqt.tile([P, T], BF16, tag=key, name=key)
                        for th in range(2):
                            ps = psum.tile([P, 512], F32, tag="mm", name="psq")
                            for co in range(NCO):
                                nc.tensor.matmul(
                                    ps[:],
                                    wq[:, co, :],
                                    xT[:, co, th * 512 : (th + 1) * 512],
                                    start=(co == 0),
                                    stop=(co == NCO - 1),
                                )
                            nc.vector.tensor_copy(
                                dst[:, th * 512 : (th + 1) * 512], ps[:]
                            )
                        packs[key] = dst
                    return packs["kt"], packs["qt"]

                kqs = {}
                def emit_pv(pk, ets):
                    for r in range(2):
                        h = 2 * pk + r
                        et = ets[r]
                        # PV with M=128 stationary window; even head
                        # [V_h | 1 | pad..] -> y rows 0:64, dn row 64;
                        # odd head shifted -64 [.. | 1(h-1) | pad | V_h] ->
                        # y rows 64:128, dn row 32 (ones col of head h-1,
                        # 32-aligned as PSUM partition offsets require)
                        wst = VS * h if r == 0 else VS * h - DK
                        dnr = DK if r == 0 else DK - 32
                        ra, rb = (0, DK) if r == 0 else (DK, P)
                        for ck in range(2):
                            a0 = ck * 512
                            b = (ck + 1) * 512
                            ytc = ytdn.tile([P, 512], F32, tag="yt", name="ytc")
                            jmax = min(NT - 1, (b - 1) // P)
                            for j in range(jmax + 1):
                                q0 = j * P
                                a = max(q0, a0)
                                esl = et[:, EOFF[j] + a - q0 : EOFF[j] + b - q0]
                                nc.tensor.matmul(
                                    ytc[:, a - a0 : b - a0],
                                    V_all[:, j, wst : wst + P],
                                    esl,
                                    start=(j == 0),
                                    stop=(j == jmax),
                                    skip_group_check=True,
                                )
                            # 1/dn from PSUM row dnr staged to a partition-0
                            # SBUF tile (custom DVE recip needs SBUF src),
                            # broadcast to all 128 rows on GpSimd (the ucode
                            # broadcasts physical partition 0), normalize ->
                            # YT pack rows
                            dnsb = attp.tile([1, 512], F32, tag="dnsb", name="dnsb")
                            nc.vector.tensor_copy(dnsb[:], ytc[dnr : dnr + 1, :])
                            rec1 = attp.tile([1, 512], F32, tag="rec1", name="rec1")
                            nc.vector.reciprocal_approx_fast(rec1[:], dnsb[:])
                            rec = attp.tile([P, 512], F32, tag="rec", name="rec")
                            nc.gpsimd.partition_broadcast(rec[:], rec1[:])
                            nc.vector.tensor_mul(
                                YT_all[ra:rb, pk, a0:b],
                                ytc[ra:rb, :],
                                rec[ra:rb, :],
                            )

                pvq = []
                if NPK:
                    kqs[0] = emit_kq(0)
                for pk in range(NPK):
                    kt, qt = kqs.pop(pk)
                    # S^T + exp for both heads, S matmuls of the pair emitted
                    # back-to-back: the two K=64 row strips (partitions 0:64
                    # and 64:128) execute concurrently on the PE sub-arrays.
                    ets = [
                        epool.tile([P, EOFF[NT]], BF16, tag="e", name="et0"),
                        epool.tile([P, EOFF[NT]], BF16, tag="e", name="et1"),
                    ]
                    for j in range(NT):
                        q0 = j * P
                        for cs in range(q0, T, 512):
                            cw = min(512, T - cs)
                            diag = cs == q0
                            sps = []
                            for r in range(2):
                                R = slice(DK * r, DK * (r + 1))
                                sp = psum.tile([P, 512], F32, tag="mm", name="sp")
                                nc.tensor.matmul(
                                    sp[:, :cw],
                                    kt[R, q0 : q0 + P],
                                    qt[R, cs : cs + cw],
                                    start=True,
                                    stop=not diag,
                                )
                                if diag:
                                    # causal mask of the diagonal block added
                                    # on the PE: psum += I^T @ mask
                                    nc.tensor.matmul(
                                        sp[:, :P],
                                        idb[:],
                                        mask[:],
                                        start=False,
                                        stop=True,
                                    )
                                sps.append(sp)
                            for r in range(2):
                                sp = sps[r]
                                o = EOFF[j] + cs - q0
                                nc.scalar.activation(
                                    ets[r][:, o : o + cw], sp[:, :cw], EXP,
                                    bias=0.0, scale=0.125,
                                )
                    if pk == 0:
                        # proj weights: queued behind pack-1 K/Q weights,
                        # long before phase C needs them
                        nc.scalar.dma_start(
                            wp0[:],
                            wproj_d[:, 0:512].rearrange("(co p) j -> p co j", p=P),
                        )
                        nc.scalar.dma_start(
                            wp1[:],
                            wproj_d[:, 512:1024].rearrange("(co p) j -> p co j", p=P),
                        )
                    if pk + 1 < NPK:
                        kqs[pk + 1] = emit_kq(pk + 1)
                    # software pipeline: PV of the PREVIOUS pack goes after
                    # this pack's S/exp and the next pack's K/Q gen, so the
                    # in-order PE queue never stalls waiting for exp(pk) --
                    # it runs kq(pk+1) + PV(pk-1) while ACT runs exp(pk).
                    if pvq:
                        emit_pv(*pvq.pop())
                    pvq.append((pk, ets))
                if pvq:
                    emit_pv(*pvq.pop())

            # ---- Phase C: output projection ----
            with tc.tile_pool(name="projp", bufs=6) as projp:
                for jc in range(2 if "C" in phases else 0):
                    wp = wp0 if jc == 0 else wp1
                    for tb in range(NT):
                        ps = psum.tile([P, 512], F32, tag="mm", name="pso")
                        for co in range(NCO):
                            nc.tensor.matmul(
                                ps[:],
                                YT_all[:, co, tb * P : (tb + 1) * P],
                                wp[:, co, :],
                                start=(co == 0),
                                stop=(co == NCO - 1),
                            )
                        # deep staging ring so the copy->DMA tail of the last
                        # chunks pipelines instead of serializing the drain
                        ot = projp.tile([P, 512], F32, tag="ot", name="ot")
                        nc.vector.tensor_copy(ot[:], ps[:])
                        eng = nc.sync if tb % 2 == 0 else nc.scalar
                        eng.dma_start(
                            y_d[tb * P : (tb + 1) * P, jc * 512 : (jc + 1) * 512],
                            ot[:],
                        )


def _get_program():
    if "nc" not in _CACHE:
        _CACHE["nc"] = _build_program()
    return _CACHE["nc"]


def make_in_maps(x, w_qkv, w_proj):
    import ml_dtypes

    x = np.asarray(x, dtype=np.float32)
    xT = np.ascontiguousarray(x.transpose(0, 2, 1))
    w_qkv = np.ascontiguousarray(np.asarray(w_qkv), dtype=np.float32)
    w_proj = np.ascontiguousarray(np.asarray(w_proj), dtype=np.float32)
    # S^T[k, q] diag-block mask: keep q >= k (upper triangle incl. diagonal)
    mask = np.where(
        np.arange(P)[None, :] >= np.arange(P)[:, None], 0.0, NEG_MASK
    ).astype(ml_dtypes.bfloat16)
    idb = np.eye(P, dtype=ml_dtypes.bfloat16)
    onesb = np.ones((P, 512), dtype=ml_dtypes.bfloat16)
    return [
        {
            "xT": xT[i],
            "w_qkv": w_qkv,
            "w_proj": w_proj,
            "mask": mask,
            "idb": idb,
            "onesb": onesb,
        }
        for i in range(NCORES)
    ]


def kernel(x, w_qkv, w_proj, _trace=False):
    from concourse.bass_utils import run_bass_kernel_spmd

    nc = _get_program()
    in_maps = make_in_maps(x, w_qkv, w_proj)
    res = run_bass_kernel_spmd(nc, in_maps, list(range(NCORES)), trace=_trace)
    out = np.stack([res.results[i]["y"] for i in range(NCORES)], axis=0)
    if _trace:
        return out, res
    return out

